# revision 1
# baseline (speedup 1.0000x reference)
"""Causal self-attention Trainium2 kernel (8-core SPMD).

Reference: y = softmax(mask(q k^T / sqrt(dh))) v -> proj, with
x [B=4, T=2048, C=1024], H=16 heads, dh=64.

Sharding: core i handles batch b = i//2 and head-group g = i%2 (8 heads).
Each core computes a partial y (its heads' contribution to the output
projection); the host sums the two partials per batch and adds proj_b.

Per-core device program (all operands pre-transposed on host so every
matmul contraction dim lands on SBUF partitions):
  phase 1: qkT[1024, T] = Wqk_loc @ x_b^T   (q rows pre-scaled by 1/8)
  phase 2: V[T, 520]    = x_b @ Wv_loc^T    (+bias; col 64 of each 65-wide
           head group is 1.0 -> PV matmul also produces softmax row-sums)
  phase 3: per head pair, per 512-query block: S^T = K_chunk @ Q^T in PSUM,
           exp on ACT (PSUM->SBUF), causal tril mask on DVE, O^T|rowsum
           accumulated via PV matmuls, normalize by DMA-broadcast 1/rowsum
  phase 4: y_partial[T, 1024] = O @ Wo_loc^T
"""

import numpy as np

C = 1024
HLOC = 8
DH = 64
QB = 512  # query block (PSUM bank width in fp32)
KC = 128  # key chunk (partition dim)

_cache = {}


def _build(T, mm_dt, dbg=False):
    import concourse.bass as bass
    import concourse.tile as tile
    from concourse import bacc, mybir

    f32 = mybir.dt.float32
    nqb = T // QB
    ctiles = C // 128
    ttiles = T // 128

    mdt = mybir.dt.float32r if mm_dt == "f32r" else f32

    def mm(ap):
        return ap

    nc = bacc.Bacc("TRN2", target_bir_lowering=False, debug=False)

    xT = nc.dram_tensor("xT", [C, T], mdt, kind="ExternalInput")
    wqkT = nc.dram_tensor("wqkT", [C // 128, 128, C // 128, 128], mdt, kind="ExternalInput")
    wvT = nc.dram_tensor("wvT", [C, 512], mdt, kind="ExternalInput")
    woT = nc.dram_tensor("woT", [512, C], mdt, kind="ExternalInput")
    qkb = nc.dram_tensor("qkb", [C], f32, kind="ExternalInput")
    vb = nc.dram_tensor("vb", [512], f32, kind="ExternalInput")
    tril = nc.dram_tensor("tril", [128, 128], f32, kind="ExternalInput")
    y = nc.dram_tensor("y", [T, C], f32, kind="ExternalOutput")
    if dbg:
        d_qkT = nc.dram_tensor("d_qkT", [128, C // 128, T], f32, kind="ExternalOutput")
        d_V = nc.dram_tensor("d_V", [128, T // 128, HLOC * 65], f32, kind="ExternalOutput")
        d_OT = nc.dram_tensor("d_OT", [128, 4, T], f32, kind="ExternalOutput")
        d_PT = nc.dram_tensor("d_PT", [128, 1024], f32, kind="ExternalOutput")
        d_PO = nc.dram_tensor("d_PO", [65, 512], f32, kind="ExternalOutput")

    Exp = mybir.ActivationFunctionType.Exp

    with tile.TileContext(nc) as tc:
        with (
            tc.tile_pool(name="persist", bufs=1) as persist,
            tc.tile_pool(name="consts", bufs=1) as consts,
        ):
            qkT_sb = persist.tile([128, ctiles, T], mdt)
            V_sb = persist.tile([128, T // 128, HLOC * 65], mdt)
            tril_sb = consts.tile([128, 128], f32)
            qkb_sb = consts.tile([128, ctiles], f32)
            vb_sb = consts.tile([128, 512], f32)

            nc.sync.dma_start(tril_sb[:], tril[:])
            nc.sync.dma_start(qkb_sb[:], qkb.rearrange("(r p) -> p r", p=128))
            vb_ap = vb[:]
            nc.sync.dma_start(
                vb_sb[:],
                bass.AP(
                    tensor=vb_ap.tensor, offset=vb_ap.offset, ap=[[0, 128], [1, 512]]
                ),
            )
            # ones columns of V (col 64 of each head's 65-wide slot).
            # memset can't produce float32r; ACT copy with scale=0, bias=1
            # (input values irrelevant but must be finite -> use tril).
            v_grp = V_sb.rearrange("p t (h c) -> p t h c", c=65)
            nc.scalar.activation(
                v_grp[:, :, :, 64:65],
                tril_sb[:, 0 : (T // 128) * HLOC].rearrange(
                    "p (a b c) -> p a b c", a=T // 128, b=HLOC, c=1
                ),
                mybir.ActivationFunctionType.Copy,
                bias=1.0,
                scale=0.0,
            )

            # ---------------- phases 1+2: projections ----------------
            with tc.tile_pool(name="pwarm", bufs=2, space="PSUM") as pwarm:
                for w in range(24):
                    wp = pwarm.tile([128, 512], f32, tag="wp", name=f"wp{w}")
                    nc.tensor.matmul(
                        wp[:], tril_sb[:], vb_sb[:], start=True, stop=True
                    )
            with (
                tc.tile_pool(name="xw", bufs=1) as xw,
                tc.tile_pool(name="wqks", bufs=2) as wqks,
                tc.tile_pool(name="pj", bufs=4, space="PSUM") as pj,
            ):
                xT_sb = xw.tile([128, ctiles, T], mdt)
                wvT_sb = xw.tile([128, ctiles, 512], mdt)
                xT_r = xT.rearrange("(c p) t -> p c t", p=128)
                for c in range(ctiles):
                    eng = (nc.sync, nc.gpsimd, nc.scalar)[c % 3]
                    eng.dma_start(xT_sb[:, c, :], xT_r[:, c, :])
                nc.gpsimd.dma_start(wvT_sb[:], wvT.rearrange("(c p) v -> p c v", p=128))

                for rt in range(ctiles):
                    wqk_t = wqks.tile([128, ctiles, 128], mdt)
                    nc.sync.dma_start(wqk_t[:], wqkT[rt])
                    for nt in range(T // 512):
                        ps = pj.tile([128, 512], f32)
                        for c in range(ctiles):
                            nc.tensor.matmul(
                                ps[:],
                                mm(wqk_t[:, c, :]),
                                mm(xT_sb[:, c, nt * 512 : (nt + 1) * 512]),
                                start=(c == 0),
                                stop=(c == ctiles - 1),
                            )
                        nc.vector.tensor_scalar_add(
                            qkT_sb[:, rt, nt * 512 : (nt + 1) * 512],
                            ps[:],
                            qkb_sb[:, rt : rt + 1],
                        )

                for tt in range(ttiles):
                    ps = pj.tile([128, 512], f32)
                    for c in range(ctiles):
                        nc.tensor.matmul(
                            ps[:],
                            mm(xT_sb[:, c, tt * 128 : (tt + 1) * 128]),
                            mm(wvT_sb[:, c, :]),
                            start=(c == 0),
                            stop=(c == ctiles - 1),
                        )
                    nc.vector.tensor_add(
                        v_grp[:, tt, :, 0:64],
                        ps.rearrange("p (h c) -> p h c", c=64),
                        vb_sb.rearrange("p (h c) -> p h c", c=64),
                    )

            if dbg:
                nc.sync.dma_start(d_qkT[:], qkT_sb[:])
                nc.sync.dma_start(d_V[:], V_sb[:])

            # ---------------- phases 3+4 ----------------
            with tc.tile_pool(name="ot", bufs=1) as ot:
                OT_sb = ot.tile([128, 4, T], mdt)
                woT_sb = ot.tile([128, 4, C], mdt)
                nc.gpsimd.dma_start(woT_sb[:], woT.rearrange("(c p) o -> p c o", p=128))

                # rowsum stash: packed on partition bases {0,32,64,96}
                # (the legal DVE bases): partition 64*lh + 32*(idx%2), col idx//2
                rs_all = ot.tile([97, (4 * nqb + 1) // 2, 512], f32)
                with (
                    tc.tile_pool(name="pexp", bufs=4) as pexp,
                    tc.tile_pool(name="rsbp", bufs=2) as rsbp,
                    tc.tile_pool(name="psS", bufs=3, space="PSUM") as psS,
                    tc.tile_pool(name="psO", bufs=2, space="PSUM") as psO,
                ):
                    for hp in range(4):
                        for qb in range(nqb):
                            po = [
                                psO.tile([65, 512], f32, tag="po", name=f"po{hp}_{qb}_{i}")
                                for i in range(2)
                            ]
                            nkc = (qb + 1) * (QB // KC)
                            q_sl = slice(qb * 512, (qb + 1) * 512)

                            # software pipeline: S-matmul pairs one step
                            # ahead of exp+PV so PE never waits on ACT
                            stage = []  # (ps, lh, kcp) pending exp+PV
                            for kcp in range(nkc // 2 + 1):
                                if kcp < nkc // 2:
                                    for lh in range(2):
                                        b0 = 64 * lh
                                        ps = psS.tile([128, 1024], f32)
                                        for j in range(2):
                                            kc = 2 * kcp + j
                                            nc.tensor.matmul(
                                                ps[:, j * 512 : (j + 1) * 512],
                                                mm(
                                                    qkT_sb[
                                                        b0 : b0 + 64,
                                                        4 + hp,
                                                        kc * 128 : (kc + 1) * 128,
                                                    ]
                                                ),
                                                mm(qkT_sb[b0 : b0 + 64, hp, q_sl]),
                                                start=True,
                                                stop=True,
                                            )
                                        stage.append((ps, lh, kcp))
                                if kcp > 0:
                                    ready, stage = stage[:2], stage[2:]
                                    for ps, lh, pkcp in ready:
                                        pt = pexp.tile([128, 1024], mdt)
                                        nc.scalar.activation(pt[:], ps[:], Exp)
                                        for j in range(2):
                                            kc = 2 * pkcp + j
                                            o = kc * 128 - qb * 512
                                            if o >= 0:
                                                nc.vector.tensor_mul(
                                                    pt[:, j * 512 + o : j * 512 + o + 128],
                                                    pt[:, j * 512 + o : j * 512 + o + 128],
                                                    tril_sb[:],
                                                )
                                            lo = max(o, 0)
                                            if dbg and hp == 0 and qb == 0 and pkcp == 0 and lh == 0 and j == 1:
                                                nc.sync.dma_start(d_PT[:], pt[:])
                                            nc.tensor.matmul(
                                                po[lh][:, lo:512],
                                                mm(
                                                    V_sb[
                                                        :,
                                                        kc,
                                                        (2 * hp + lh) * 65 : (2 * hp + lh) * 65
                                                        + 65,
                                                    ]
                                                ),
                                                mm(pt[:, j * 512 + lo : (j + 1) * 512]),
                                                start=(kc == 0),
                                                stop=(kc == nkc - 1),
                                            )

                            if dbg and hp == 0 and qb == 0:
                                po_cp = rsbp.tile([65, 512], f32, tag="pocp", name="po_cp")
                                nc.vector.tensor_copy(po_cp[:], po[0][:])
                                nc.sync.dma_start(d_PO[:], po_cp[:])
                            # stash rowsums + unnormalized O^T (pure DVE, no
                            # DMA in the PSUM-release path); normalization is
                            # batched after the attention loops
                            idx = hp * nqb + qb
                            for lh in range(2):
                                p0 = 64 * lh + 32 * (idx % 2)
                                nc.vector.tensor_copy(
                                    rs_all[p0 : p0 + 1, idx // 2, :],
                                    po[lh][64:65, :],
                                )
                                nc.vector.tensor_copy(
                                    OT_sb[64 * lh : 64 * lh + 64, hp, q_sl],
                                    po[lh][0:64, :],
                                )

                        # ---- per-hp batched 1/rowsum + normalize ----
                        # (overlaps with next head pair's attention)
                        rsc = rsbp.tile(
                            [128, max(nqb, 2) * 256 // 16],
                            f32,
                            tag="rsc",
                            name=f"rsc{hp}",
                        )
                        nc.vector.memset(rsc[:, :], 1.0)
                        groups = []  # (lh, par, col0, ncols)
                        for lh in range(2):
                            for par in range(2):
                                idxs = [
                                    hp * nqb + qb
                                    for qb in range(nqb)
                                    if (hp * nqb + qb) % 2 == par
                                ]
                                if idxs:
                                    groups.append(
                                        (lh, par, min(i // 2 for i in idxs), len(idxs))
                                    )
                        def _gflats(lh, par, col0, ncols):
                            p0 = 64 * lh + 32 * par
                            row = rs_all[p0 : p0 + 1, col0 : col0 + ncols, :]
                            n = ncols * 512
                            flat = bass.AP(
                                tensor=row.tensor,
                                offset=row.offset,
                                ap=[list(row.ap[0]), [1, n]],
                            )
                            rid = 32 * (2 * lh + par)
                            qr = rsc[rid : rid + 32, 0 : n // 32]
                            qflat = bass.AP(
                                tensor=qr.tensor,
                                offset=qr.offset,
                                ap=[list(qr.ap[0]), [1, n // 32]],
                            )
                            return flat, qflat
                        for grp in groups:
                            flat, qflat = _gflats(*grp)
                            nc.sync.dma_start(qflat, flat)
                        nc.vector.reciprocal(rsc[:, :], rsc[:, :])
                        for grp in groups:
                            flat, qflat = _gflats(*grp)
                            nc.sync.dma_start(flat, qflat)
                        for qb in range(nqb):
                            idx = hp * nqb + qb
                            q_sl = slice(qb * 512, (qb + 1) * 512)
                            rsb = rsbp.tile([128, 512], f32)
                            for lh in range(2):
                                p0 = 64 * lh + 32 * (idx % 2)
                                half = rs_all[p0 : p0 + 1, idx // 2, :]
                                src = bass.AP(
                                    tensor=half.tensor,
                                    offset=half.offset,
                                    ap=[list(half.ap[0]), [0, 64], [1, 512]],
                                )
                                nc.sync.dma_start(rsb[64 * lh : 64 * lh + 64, :], src)
                            for lh in range(2):
                                b0 = 64 * lh
                                nc.vector.tensor_mul(
                                    OT_sb[b0 : b0 + 64, hp, q_sl],
                                    OT_sb[b0 : b0 + 64, hp, q_sl],
                                    rsb[b0 : b0 + 64, :],
                                )

                if dbg:
                    nc.sync.dma_start(d_OT[:], OT_sb[:])

                # -------- phase 4: output projection --------
                with (
                    tc.tile_pool(name="yp", bufs=4) as yp,
                    tc.tile_pool(name="pj2", bufs=4, space="PSUM") as pj2,
                ):
                    for tt in range(ttiles):
                        for nt in range(2):
                            ps = pj2.tile([128, 512], f32)
                            for c4 in range(4):
                                nc.tensor.matmul(
                                    ps[:],
                                    mm(OT_sb[:, c4, tt * 128 : (tt + 1) * 128]),
                                    mm(woT_sb[:, c4, nt * 512 : (nt + 1) * 512]),
                                    start=(c4 == 0),
                                    stop=(c4 == 3),
                                )
                            yt = yp.tile([128, 512], f32)
                            nc.vector.tensor_copy(yt[:], ps[:])
                            nc.sync.dma_start(
                                y[tt * 128 : (tt + 1) * 128, nt * 512 : (nt + 1) * 512],
                                yt[:],
                            )

    nc.compile()
    return nc


def get_nc(T=2048, mm_dt="f32r", dbg=False):
    key = (T, mm_dt, dbg)
    if key not in _cache:
        _cache[key] = _build(T, mm_dt, dbg)
    return _cache[key]


def make_in_maps(x, qkv_w, qkv_b, proj_w, proj_b):
    B, T, _ = x.shape
    f = np.float32
    # S^T blocks are [key, query]: keep k <= q  ->  upper triangle
    tril = np.triu(np.ones((128, 128), f))
    in_maps = []
    for i in range(B * 2):
        b, g = i // 2, i % 2
        sl = slice(g * 512, (g + 1) * 512)
        wq = qkv_w[0 * C : 1 * C][sl] * (1.0 / 8.0)
        wk = qkv_w[1 * C : 2 * C][sl]
        wv = qkv_w[2 * C : 3 * C][sl]
        in_maps.append(
            {
                "xT": np.ascontiguousarray(x[b].T, f),
                "wqkT": np.ascontiguousarray(
                    np.stack(
                        [
                            np.concatenate([wq, wk], 0)
                            .T[:, rt * 128 : (rt + 1) * 128]
                            .reshape(C // 128, 128, 128)
                            .transpose(1, 0, 2)
                            for rt in range(C // 128)
                        ]
                    ),
                    f,
                ),
                "wvT": np.ascontiguousarray(wv.T, f),
                "woT": np.ascontiguousarray(proj_w[:, sl].T, f),
                "qkb": np.concatenate(
                    [qkv_b[0 * C : 1 * C][sl] * (1.0 / 8.0), qkv_b[1 * C : 2 * C][sl]]
                ).astype(f),
                "vb": qkv_b[2 * C : 3 * C][sl].astype(f),
                "tril": tril,
            }
        )
    return in_maps


def kernel(x, qkv_w, qkv_b, proj_w, proj_b, mm_dt="f32r", trace=False, tmpdir=None):
    from concourse.bass_utils import run_bass_kernel_spmd

    x = np.asarray(x, np.float32)
    qkv_w = np.asarray(qkv_w, np.float32)
    qkv_b = np.asarray(qkv_b, np.float32)
    proj_w = np.asarray(proj_w, np.float32)
    proj_b = np.asarray(proj_b, np.float32)

    B, T, _ = x.shape
    nc = get_nc(T, mm_dt)
    in_maps = make_in_maps(x, qkv_w, qkv_b, proj_w, proj_b)
    res = run_bass_kernel_spmd(
        nc, in_maps, list(range(len(in_maps))), trace=trace, tmpdir=tmpdir
    )
    out = np.empty((B, T, C), np.float32)
    for b in range(B):
        out[b] = res.results[2 * b]["y"] + res.results[2 * b + 1]["y"] + proj_b
    kernel.last_result = res
    return out



# revision 8
# speedup vs baseline: 1.2060x; 1.2060x over previous
"""Causal self-attention Trainium2 kernel (8-core SPMD), v2.

Reference: y = softmax(mask(q k^T / sqrt(dh))) v -> proj, with
x [B=4, T=2048, C=1024], H=16 heads, dh=64.

Sharding: core i handles batch b = i//2 and head-group g = i%2 (8 heads).
Each core computes a partial y (its heads' contribution to the output
projection); the host sums the two partials per batch and adds proj_b.

v2 design (vs v1): all matmul operands in bf16 (2 cols/cycle PE stream +
fast weight load), true-causal trimming of S/exp/PV, the two heads of a
pair share one exp instruction, and the whole kernel is a single
software-pipelined stream: QKV/V projections, attention, normalization
and the output projection are interleaved so the PE fills the gaps left
by the ACT-bound exp stream.

Per-core schedule:
  startup: const DMAs, weight DMAs, x columns, PE warmup (HAM
           un-throttle), exp-table preload, V ones-columns, projections
           for round 0
  round r (query block of 512 queries):
    normalize round r-1 (reciprocal + OT muls), then per head pair hp:
      pipelined kc chunks: S^T (PSUM [128, 2x512], trimmed), exp (ACT,
      one inst both heads), tril mask (DVE, diagonal), PV accumulate
      (PSUM [65,512]; V ones-column makes row 64 the softmax rowsum)
      + 1 filler tile per kc step: QK projection (round r+1), V tiles,
      output projection (round r-1)
      po evac: DVE [65,512] PSUM -> stage SBUF bf16; rowsum row
      broadcast-DMA'd into rsb [64,512]
  tail: normalize + output projection of the last round
"""

import numpy as np

C = 1024
HLOC = 8
DH = 64
QB = 512  # query block
KC = 128  # key chunk (PSUM partition dim)

_cache = {}


def _build(T, dbg=False):
    import concourse.bass as bass
    import concourse.tile as tile
    from concourse import bacc, mybir

    f32 = mybir.dt.float32
    bf16 = mybir.dt.bfloat16
    nqb = T // QB          # 4 rounds
    ctiles = C // 128      # 8
    ttiles = T // 128      # 16

    nc = bacc.Bacc("TRN2", target_bir_lowering=False, debug=False)

    xT = nc.dram_tensor("xT", [C, T], bf16, kind="ExternalInput")
    wqkT = nc.dram_tensor("wqkT", [128, ctiles, ctiles, 128], bf16, kind="ExternalInput")
    wvT = nc.dram_tensor("wvT", [C, 512], bf16, kind="ExternalInput")
    woT = nc.dram_tensor("woT", [512, C], bf16, kind="ExternalInput")
    qkb = nc.dram_tensor("qkb", [C], f32, kind="ExternalInput")
    vb = nc.dram_tensor("vb", [512], bf16, kind="ExternalInput")
    tril = nc.dram_tensor("tril", [128, 128], bf16, kind="ExternalInput")
    y = nc.dram_tensor("y", [T, C], f32, kind="ExternalOutput")
    if dbg:
        d_qkT = nc.dram_tensor("d_qkT", [128, ctiles, T], bf16, kind="ExternalOutput")
        d_V = nc.dram_tensor("d_V", [128, ttiles, HLOC * 65], bf16, kind="ExternalOutput")
        d_OT = nc.dram_tensor("d_OT", [128, 4, T], bf16, kind="ExternalOutput")

    Exp = mybir.ActivationFunctionType.Exp
    Copy = mybir.ActivationFunctionType.Copy

    with nc.allow_low_precision("attention tolerates bf16 (rel tol 2e-2)"):
        with tile.TileContext(nc) as tc:
            with (
                tc.tile_pool(name="persist", bufs=1) as persist,
                tc.tile_pool(name="consts", bufs=1) as consts,
                tc.tile_pool(name="stage", bufs=16) as stagep,
                tc.tile_pool(name="pexp", bufs=3) as pexp,
                tc.tile_pool(name="yp", bufs=4) as yp,
                tc.tile_pool(name="pj", bufs=2, space="PSUM") as pj,
                tc.tile_pool(name="psS", bufs=2, space="PSUM") as psS,
                tc.tile_pool(name="psO", bufs=2, space="PSUM") as psO,
            ):
                qkT_sb = persist.tile([128, ctiles, T], bf16)
                V_sb = persist.tile([128, ttiles, HLOC * 65], bf16)
                xT_sb = persist.tile([128, ctiles, T], bf16)
                wqk_sb = persist.tile([128, ctiles, ctiles, 128], bf16)
                wvT_sb = persist.tile([128, ctiles, 512], bf16)
                woT_sb = persist.tile([128, 4, C], bf16)
                OT_sb = persist.tile([128, 4, T], bf16)
                rsb = persist.tile([128, 4, QB], bf16)
                tril_sb = consts.tile([128, 128], bf16)
                qkb_sb = consts.tile([128, ctiles], f32)
                vb_sb = consts.tile([128, 512], bf16)
                scr = consts.tile([1, 8], f32)

                # ---- startup DMAs ----
                nc.sync.dma_start(tril_sb[:], tril[:])
                nc.sync.dma_start(qkb_sb[:], qkb.rearrange("(r p) -> p r", p=128))
                vb_ap = vb[:]
                nc.sync.dma_start(
                    vb_sb[:],
                    bass.AP(
                        tensor=vb_ap.tensor, offset=vb_ap.offset, ap=[[0, 128], [1, 512]]
                    ),
                )
                nc.gpsimd.dma_start(wqk_sb[:], wqkT[:])
                nc.gpsimd.dma_start(wvT_sb[:], wvT.rearrange("(c p) v -> p c v", p=128))
                nc.gpsimd.dma_start(woT_sb[:], woT.rearrange("(c p) o -> p c o", p=128))
                xT_r = xT.rearrange("(c p) t -> p c t", p=128)
                for nt in range(nqb):
                    for c in range(ctiles):
                        nc.sync.dma_start(
                            xT_sb[:, c, nt * 512 : (nt + 1) * 512],
                            xT_r[:, c, nt * 512 : (nt + 1) * 512],
                        )

                # exp table preload (off the hot path)
                nc.scalar.activation(scr[:, 0:8], tril_sb[0:1, 0:8], Exp)

                # V ones-columns (col 64 of each head's 65-wide slot)
                v_grp = V_sb.rearrange("p t (h c) -> p t h c", c=65)
                nc.scalar.activation(
                    v_grp[:, :, :, 64:65],
                    tril_sb[:, 0 : ttiles * HLOC].rearrange(
                        "p (a b c) -> p a b c", a=ttiles, b=HLOC, c=1
                    ),
                    Copy,
                    bias=1.0,
                    scale=0.0,
                )

                # ---- PE warmup: un-throttle HAM (~5us of dummy matmuls) ----
                for w in range(24):
                    wp = pj.tile([128, 512], f32, tag="pj", name=f"wp{w}")
                    nc.tensor.matmul(wp[:], tril_sb[:], vb_sb[:], start=True, stop=True)

                # ---------- filler emission machinery ----------
                def emit_proj(rt, nt):
                    """One QK-projection output tile [128, 512] -> qkT_sb."""
                    ps = pj.tile([128, 512], f32, tag="pj", name=f"pj{rt}_{nt}")
                    for c in range(ctiles):
                        nc.tensor.matmul(
                            ps[:],
                            wqk_sb[:, rt, c, :],
                            xT_sb[:, c, nt * 512 : (nt + 1) * 512],
                            start=(c == 0),
                            stop=(c == ctiles - 1),
                        )
                    nc.vector.tensor_scalar_add(
                        qkT_sb[:, rt, nt * 512 : (nt + 1) * 512],
                        ps[:],
                        qkb_sb[:, rt : rt + 1],
                    )

                def emit_v(tt):
                    """One V tile [128 keys, 512 v-cols] -> V_sb (+bias)."""
                    ps = pj.tile([128, 512], f32, tag="pj", name=f"pv{tt}")
                    for c in range(ctiles):
                        nc.tensor.matmul(
                            ps[:],
                            xT_sb[:, c, tt * 128 : (tt + 1) * 128],
                            wvT_sb[:, c, :],
                            start=(c == 0),
                            stop=(c == ctiles - 1),
                        )
                    nc.vector.tensor_add(
                        v_grp[:, tt, :, 0:64],
                        ps.rearrange("p (h c) -> p h c", c=64),
                        vb_sb.rearrange("p (h c) -> p h c", c=64),
                    )

                def emit_y(tt, nt):
                    """One output-projection tile y[tt*128:, nt*512:]."""
                    ps = pj.tile([128, 512], f32, tag="pj", name=f"py{tt}_{nt}")
                    for c4 in range(4):
                        nc.tensor.matmul(
                            ps[:],
                            OT_sb[:, c4, tt * 128 : (tt + 1) * 128],
                            woT_sb[:, c4, nt * 512 : (nt + 1) * 512],
                            start=(c4 == 0),
                            stop=(c4 == 3),
                        )
                    yt = yp.tile([128, 512], f32, tag="yt", name=f"yt{tt}_{nt}")
                    nc.vector.tensor_copy(yt[:], ps[:])
                    nc.sync.dma_start(
                        y[tt * 128 : (tt + 1) * 128, nt * 512 : (nt + 1) * 512],
                        yt[:],
                    )

                filler = []

                def pop_filler(n=1):
                    for _ in range(n):
                        if filler:
                            filler.pop(0)()

                # pre-round work: projections for round 0 (K first), V 0..3
                for hp in range(4):
                    emit_proj(4 + hp, 0)
                for hp in range(4):
                    emit_proj(hp, 0)
                for tt in range(4):
                    emit_v(tt)

                # ---------- main rounds ----------
                stages = {}  # (hp, lh) -> stage tile of current round

                def attention(hp, qb, rs_rows):
                    nkc = 4 * (qb + 1)
                    q0 = qb * 512
                    po = [
                        psO.tile([65, 512], f32, tag="po", name=f"po{hp}_{qb}_{i}")
                        for i in range(2)
                    ]
                    pend = []  # psS tiles awaiting exp+PV: (ps, kc)

                    def do_exp_pv(ps, kc):
                        lo = max(kc * 128 - q0, 0)
                        ps3 = ps.rearrange("p (l q) -> p l q", l=2)
                        pt = pexp.tile([128, 2, 512], bf16, tag="pt", name=f"pt{hp}_{qb}_{kc}")
                        nc.scalar.activation(
                            pt[:, :, lo:512], ps3[:, :, lo:512], Exp
                        )
                        if kc * 128 >= q0:  # diagonal chunk: tril mask
                            for lh in range(2):
                                nc.vector.tensor_mul(
                                    pt[:, lh, lo : lo + 128],
                                    pt[:, lh, lo : lo + 128],
                                    tril_sb[:],
                                )
                        for lh in range(2):
                            nc.tensor.matmul(
                                po[lh][:, lo:512],
                                V_sb[:, kc, (2 * hp + lh) * 65 : (2 * hp + lh) * 65 + 65],
                                pt[:, lh, lo:512],
                                start=(kc == 0),
                                stop=(kc == nkc - 1),
                            )

                    for kc in range(nkc):
                        lo = max(kc * 128 - q0, 0)
                        ps = psS.tile([128, 1024], f32, tag="ps", name=f"ps{hp}_{qb}_{kc}")
                        ps3 = ps.rearrange("p (l q) -> p l q", l=2)
                        for lh in range(2):
                            b0 = 64 * lh
                            nc.tensor.matmul(
                                ps3[:, lh, lo:512],
                                qkT_sb[b0 : b0 + 64, 4 + hp, kc * 128 : (kc + 1) * 128],
                                qkT_sb[b0 : b0 + 64, hp, q0 + lo : q0 + 512],
                                start=True,
                                stop=True,
                            )
                        pend.append((ps, kc))
                        if len(pend) > 1:
                            do_exp_pv(*pend.pop(0))
                        pop_filler(1)
                    do_exp_pv(*pend.pop(0))

                    # evacuate po: PSUM -> stage (bf16, lh halves base-aligned
                    # with OT); rowsum rows -> rs_rows slots, bcast into rsb
                    st = stagep.tile([128, 512], bf16, tag="st", name=f"st{hp}_{qb}")
                    for lh in range(2):
                        s = 2 * hp + lh
                        nc.vector.tensor_copy(
                            st[64 * lh : 64 * lh + 64, :], po[lh][0:64, :]
                        )
                        nc.vector.tensor_copy(
                            rs_rows[32 * (s % 4) : 32 * (s % 4) + 1, s // 4, :],
                            po[lh][64:65, :],
                        )
                        row = rs_rows[32 * (s % 4) : 32 * (s % 4) + 1, s // 4, :]
                        nc.gpsimd.dma_start(
                            rsb[64 * lh : 64 * lh + 64, hp, :],
                            bass.AP(
                                tensor=row.tensor,
                                offset=row.offset,
                                ap=[list(row.ap[0]), [0, 64], [1, 512]],
                            ),
                        )
                    stages[hp] = st

                def emit_norm(qb, stg):
                    """Normalize round qb's OT from stage tiles + rsb."""
                    q_sl = slice(qb * 512, (qb + 1) * 512)
                    rflat = rsb.rearrange("p h q -> p (h q)")
                    nc.vector.reciprocal(rflat[:, :], rflat[:, :])
                    for hp in range(4):
                        nc.vector.tensor_mul(
                            OT_sb[:, hp, q_sl],
                            stg[hp][:, :],
                            rsb[:, hp, :],
                        )

                for qb in range(nqb):
                    if qb >= 1:
                        emit_norm(qb - 1, dict(stages))
                    # filler for this round: projections + V for round qb+1,
                    # output projection of round qb-1 (normalized just above)
                    if qb + 1 < nqb:
                        for hp in range(4):
                            filler.append(
                                lambda rt=4 + hp, nt=qb + 1: emit_proj(rt, nt)
                            )
                        for hp in range(4):
                            filler.append(lambda rt=hp, nt=qb + 1: emit_proj(rt, nt))
                        for tt in range(4 * qb + 4, 4 * qb + 8):
                            filler.append(lambda tt=tt: emit_v(tt))
                    if qb >= 1:
                        for tt in range(4 * (qb - 1), 4 * qb):
                            for nt in range(2):
                                filler.append(lambda tt=tt, nt=nt: emit_y(tt, nt))
                    rs_rows = stagep.tile(
                        [97, 2, 512], bf16, tag="rs", bufs=2, name=f"rs{qb}"
                    )
                    for hp in range(4):
                        attention(hp, qb, rs_rows)

                # tail: drain filler, normalize last round, project it
                pop_filler(len(filler))
                emit_norm(nqb - 1, dict(stages))
                for tt in range(4 * (nqb - 1), 4 * nqb):
                    for nt in range(2):
                        emit_y(tt, nt)

                if dbg:
                    nc.sync.dma_start(d_qkT[:], qkT_sb[:])
                    nc.sync.dma_start(d_V[:], V_sb[:])
                    nc.sync.dma_start(d_OT[:], OT_sb[:])

    nc.compile()
    return nc


def get_nc(T=2048, mm_dt="bf16", dbg=False):
    key = (T, dbg)
    if key not in _cache:
        _cache[key] = _build(T, dbg)
    return _cache[key]


def make_in_maps(x, qkv_w, qkv_b, proj_w, proj_b):
    import ml_dtypes

    bf = ml_dtypes.bfloat16
    B, T, _ = x.shape
    f = np.float32
    # S^T blocks are [key, query]: keep k <= q  ->  upper triangle
    tril = np.triu(np.ones((128, 128), f)).astype(bf)
    in_maps = []
    for i in range(B * 2):
        b, g = i // 2, i % 2
        sl = slice(g * 512, (g + 1) * 512)
        wq = qkv_w[0 * C : 1 * C][sl] * (1.0 / 8.0)
        wk = qkv_w[1 * C : 2 * C][sl]
        wv = qkv_w[2 * C : 3 * C][sl]
        # [rt, 128p, c, 128] -> partition-major [128p, rt, c, 128]
        wqk4 = np.stack(
            [
                np.concatenate([wq, wk], 0)
                .T[:, rt * 128 : (rt + 1) * 128]
                .reshape(C // 128, 128, 128)
                .transpose(1, 0, 2)
                for rt in range(C // 128)
            ]
        ).transpose(1, 0, 2, 3)
        in_maps.append(
            {
                "xT": np.ascontiguousarray(x[b].T).astype(bf),
                "wqkT": np.ascontiguousarray(wqk4).astype(bf),
                "wvT": np.ascontiguousarray(wv.T).astype(bf),
                "woT": np.ascontiguousarray(proj_w[:, sl].T).astype(bf),
                "qkb": np.concatenate(
                    [qkv_b[0 * C : 1 * C][sl] * (1.0 / 8.0), qkv_b[1 * C : 2 * C][sl]]
                ).astype(f),
                "vb": qkv_b[2 * C : 3 * C][sl].astype(bf),
                "tril": tril,
            }
        )
    return in_maps


def kernel(x, qkv_w, qkv_b, proj_w, proj_b, mm_dt="bf16", trace=False, tmpdir=None):
    from concourse.bass_utils import run_bass_kernel_spmd

    x = np.asarray(x, np.float32)
    qkv_w = np.asarray(qkv_w, np.float32)
    qkv_b = np.asarray(qkv_b, np.float32)
    proj_w = np.asarray(proj_w, np.float32)
    proj_b = np.asarray(proj_b, np.float32)

    B, T, _ = x.shape
    nc = get_nc(T)
    in_maps = make_in_maps(x, qkv_w, qkv_b, proj_w, proj_b)
    res = run_bass_kernel_spmd(
        nc, in_maps, list(range(len(in_maps))), trace=trace, tmpdir=tmpdir
    )
    out = np.empty((B, T, C), np.float32)
    for b in range(B):
        out[b] = res.results[2 * b]["y"] + res.results[2 * b + 1]["y"] + proj_b
    kernel.last_result = res
    return out


# revision 10
# speedup vs baseline: 1.2831x; 1.0640x over previous
"""Causal self-attention Trainium2 kernel (8-core SPMD), v2.

Reference: y = softmax(mask(q k^T / sqrt(dh))) v -> proj, with
x [B=4, T=2048, C=1024], H=16 heads, dh=64.

Sharding: core i handles batch b = i//2 and head-group g = i%2 (8 heads).
Each core computes a partial y (its heads' contribution to the output
projection); the host sums the two partials per batch and adds proj_b.

v2 design (vs v1): all matmul operands in bf16 (2 cols/cycle PE stream +
fast weight load), true-causal trimming of S/exp/PV, the two heads of a
pair share one exp instruction, and the whole kernel is a single
software-pipelined stream: QKV/V projections, attention, normalization
and the output projection are interleaved so the PE fills the gaps left
by the ACT-bound exp stream.

Per-core schedule:
  startup: const DMAs, weight DMAs, x columns, PE warmup (HAM
           un-throttle), exp-table preload, V ones-columns, projections
           for round 0
  round r (query block of 512 queries):
    normalize round r-1 (reciprocal + OT muls), then per head pair hp:
      pipelined kc chunks: S^T (PSUM [128, 2x512], trimmed), exp (ACT,
      one inst both heads), tril mask (DVE, diagonal), PV accumulate
      (PSUM [65,512]; V ones-column makes row 64 the softmax rowsum)
      + 1 filler tile per kc step: QK projection (round r+1), V tiles,
      output projection (round r-1)
      po evac: DVE [65,512] PSUM -> stage SBUF bf16; rowsum row
      broadcast-DMA'd into rsb [64,512]
  tail: normalize + output projection of the last round
"""

import numpy as np

C = 1024
HLOC = 8
DH = 64
QB = 512  # query block
KC = 128  # key chunk (PSUM partition dim)

_cache = {}


def _build(T, dbg=False):
    import concourse.bass as bass
    import concourse.tile as tile
    from concourse import bacc, mybir

    f32 = mybir.dt.float32
    bf16 = mybir.dt.bfloat16
    nqb = T // QB          # 4 rounds
    ctiles = C // 128      # 8
    ttiles = T // 128      # 16

    nc = bacc.Bacc("TRN2", target_bir_lowering=False, debug=False)

    xT = nc.dram_tensor("xT", [C, T], bf16, kind="ExternalInput")
    wqkT = nc.dram_tensor("wqkT", [128, ctiles, ctiles, 128], bf16, kind="ExternalInput")
    wvT = nc.dram_tensor("wvT", [C, 512], bf16, kind="ExternalInput")
    woT = nc.dram_tensor("woT", [512, C], bf16, kind="ExternalInput")
    qkb = nc.dram_tensor("qkb", [C], f32, kind="ExternalInput")
    vb = nc.dram_tensor("vb", [512], bf16, kind="ExternalInput")
    tril = nc.dram_tensor("tril", [128, 128], bf16, kind="ExternalInput")
    y = nc.dram_tensor("y", [T, C], f32, kind="ExternalOutput")
    if dbg:
        d_qkT = nc.dram_tensor("d_qkT", [128, ctiles, T], bf16, kind="ExternalOutput")
        d_V = nc.dram_tensor("d_V", [128, ttiles, HLOC * 65], bf16, kind="ExternalOutput")
        d_OT = nc.dram_tensor("d_OT", [128, 4, T], bf16, kind="ExternalOutput")

    Exp = mybir.ActivationFunctionType.Exp
    Copy = mybir.ActivationFunctionType.Copy

    with nc.allow_low_precision("attention tolerates bf16 (rel tol 2e-2)"):
        with tile.TileContext(nc) as tc:
            with (
                tc.tile_pool(name="persist", bufs=1) as persist,
                tc.tile_pool(name="consts", bufs=1) as consts,
                tc.tile_pool(name="stage", bufs=16) as stagep,
                tc.tile_pool(name="pexp", bufs=3) as pexp,
                tc.tile_pool(name="yp", bufs=4) as yp,
                tc.tile_pool(name="pj", bufs=2, space="PSUM") as pj,
                tc.tile_pool(name="psS", bufs=2, space="PSUM") as psS,
                tc.tile_pool(name="psO", bufs=2, space="PSUM") as psO,
            ):
                qkT_sb = persist.tile([128, ctiles, T], bf16)
                V_sb = persist.tile([128, ttiles, HLOC * 65], bf16)
                xT_sb = persist.tile([128, ctiles, T], bf16)
                wqk_sb = persist.tile([128, ctiles, ctiles, 128], bf16)
                wvT_sb = persist.tile([128, ctiles, 512], bf16)
                woT_sb = persist.tile([128, 4, C], bf16)
                OT_sb = persist.tile([128, 4, T], bf16)
                rsb = persist.tile([128, 4, QB], bf16)
                tril_sb = consts.tile([128, 128], bf16)
                qkb_sb = consts.tile([128, ctiles], f32)
                vb_sb = consts.tile([128, 512], bf16)
                scr = consts.tile([1, 8], f32)

                # ---- startup DMAs (spread across queues/engines) ----
                nc.sync.dma_start(tril_sb[:], tril[:])
                nc.scalar.dma_start(qkb_sb[:], qkb.rearrange("(r p) -> p r", p=128))
                vb_ap = vb[:]
                nc.scalar.dma_start(
                    vb_sb[:],
                    bass.AP(
                        tensor=vb_ap.tensor, offset=vb_ap.offset, ap=[[0, 128], [1, 512]]
                    ),
                )
                nc.gpsimd.dma_start(wqk_sb[:], wqkT[:])
                nc.gpsimd.dma_start(wvT_sb[:], wvT.rearrange("(c p) v -> p c v", p=128))
                nc.gpsimd.dma_start(woT_sb[:], woT.rearrange("(c p) o -> p c o", p=128))
                xT_r = xT.rearrange("(c p) t -> p c t", p=128)

                def dma_x(nt, c, eng):
                    eng.dma_start(
                        xT_sb[:, c, nt * 512 : (nt + 1) * 512],
                        xT_r[:, c, nt * 512 : (nt + 1) * 512],
                    )

                for c in range(4):
                    dma_x(0, c, nc.sync)
                for c in range(4, 8):
                    dma_x(0, c, nc.scalar)
                for c in range(4):
                    dma_x(1, c, nc.sync)
                for c in range(4, 8):
                    dma_x(1, c, nc.scalar)
                for nt in (2, 3):
                    for c in range(ctiles):
                        dma_x(nt, c, nc.gpsimd)

                # exp table preload (off the hot path)
                nc.scalar.activation(scr[:, 0:8], tril_sb[0:1, 0:8], Exp)

                # V ones-columns (col 64 of each head's 65-wide slot)
                v_grp = V_sb.rearrange("p t (h c) -> p t h c", c=65)
                nc.scalar.activation(
                    v_grp[:, :, :, 64:65],
                    tril_sb[:, 0 : ttiles * HLOC].rearrange(
                        "p (a b c) -> p a b c", a=ttiles, b=HLOC, c=1
                    ),
                    Copy,
                    bias=1.0,
                    scale=0.0,
                )

                # ---- PE warmup: un-throttle HAM (~4.5us of dummy matmuls),
                # depends only on tril (first DMA)
                for w in range(40):
                    wp = pj.tile([128, 512], f32, tag="pj", name=f"wp{w}")
                    nc.tensor.matmul(
                        wp[:, 0:128], tril_sb[:], tril_sb[:], start=True, stop=True
                    )

                # ---------- filler emission machinery ----------
                def emit_proj(rt, nt):
                    """One QK-projection output tile [128, 512] -> qkT_sb."""
                    ps = pj.tile([128, 512], f32, tag="pj", name=f"pj{rt}_{nt}")
                    for c in range(ctiles):
                        nc.tensor.matmul(
                            ps[:],
                            wqk_sb[:, rt, c, :],
                            xT_sb[:, c, nt * 512 : (nt + 1) * 512],
                            start=(c == 0),
                            stop=(c == ctiles - 1),
                        )
                    nc.vector.tensor_scalar_add(
                        qkT_sb[:, rt, nt * 512 : (nt + 1) * 512],
                        ps[:],
                        qkb_sb[:, rt : rt + 1],
                    )

                def emit_v(tt):
                    """One V tile [128 keys, 512 v-cols] -> V_sb (+bias)."""
                    ps = pj.tile([128, 512], f32, tag="pj", name=f"pv{tt}")
                    for c in range(ctiles):
                        nc.tensor.matmul(
                            ps[:],
                            xT_sb[:, c, tt * 128 : (tt + 1) * 128],
                            wvT_sb[:, c, :],
                            start=(c == 0),
                            stop=(c == ctiles - 1),
                        )
                    nc.vector.tensor_add(
                        v_grp[:, tt, :, 0:64],
                        ps.rearrange("p (h c) -> p h c", c=64),
                        vb_sb.rearrange("p (h c) -> p h c", c=64),
                    )

                def emit_y(tt, nt):
                    """One output-projection tile y[tt*128:, nt*512:]."""
                    ps = pj.tile([128, 512], f32, tag="pj", name=f"py{tt}_{nt}")
                    for c4 in range(4):
                        nc.tensor.matmul(
                            ps[:],
                            OT_sb[:, c4, tt * 128 : (tt + 1) * 128],
                            woT_sb[:, c4, nt * 512 : (nt + 1) * 512],
                            start=(c4 == 0),
                            stop=(c4 == 3),
                        )
                    yt = yp.tile([128, 512], f32, tag="yt", name=f"yt{tt}_{nt}")
                    nc.vector.tensor_copy(yt[:], ps[:])
                    nc.sync.dma_start(
                        y[tt * 128 : (tt + 1) * 128, nt * 512 : (nt + 1) * 512],
                        yt[:],
                    )

                filler = []

                def pop_filler(n=1):
                    for _ in range(n):
                        if filler:
                            filler.pop(0)()

                # pre-round work: projections for round 0 (K first), V 0..3
                for hp in range(4):
                    emit_proj(4 + hp, 0)
                for hp in range(4):
                    emit_proj(hp, 0)
                for tt in range(4):
                    emit_v(tt)

                # ---------- main rounds ----------
                stages = {}  # (hp, lh) -> stage tile of current round

                def attention(hp, qb, rs_rows):
                    nkc = 4 * (qb + 1)
                    q0 = qb * 512
                    po = [
                        psO.tile([65, 512], f32, tag="po", name=f"po{hp}_{qb}_{i}")
                        for i in range(2)
                    ]
                    pend = []  # psS tiles awaiting exp+PV: (ps, kc)

                    def do_exp_pv(ps, kc):
                        lo = max(kc * 128 - q0, 0)
                        ps3 = ps.rearrange("p (l q) -> p l q", l=2)
                        pt = pexp.tile([128, 2, 512], bf16, tag="pt", name=f"pt{hp}_{qb}_{kc}")
                        nc.scalar.activation(
                            pt[:, :, lo:512], ps3[:, :, lo:512], Exp
                        )
                        if kc * 128 >= q0:  # diagonal chunk: tril mask
                            for lh in range(2):
                                nc.vector.tensor_mul(
                                    pt[:, lh, lo : lo + 128],
                                    pt[:, lh, lo : lo + 128],
                                    tril_sb[:],
                                )
                        for lh in range(2):
                            nc.tensor.matmul(
                                po[lh][:, lo:512],
                                V_sb[:, kc, (2 * hp + lh) * 65 : (2 * hp + lh) * 65 + 65],
                                pt[:, lh, lo:512],
                                start=(kc == 0),
                                stop=(kc == nkc - 1),
                            )

                    for kc in range(nkc):
                        lo = max(kc * 128 - q0, 0)
                        ps = psS.tile([128, 1024], f32, tag="ps", name=f"ps{hp}_{qb}_{kc}")
                        ps3 = ps.rearrange("p (l q) -> p l q", l=2)
                        for lh in range(2):
                            b0 = 64 * lh
                            nc.tensor.matmul(
                                ps3[:, lh, lo:512],
                                qkT_sb[b0 : b0 + 64, 4 + hp, kc * 128 : (kc + 1) * 128],
                                qkT_sb[b0 : b0 + 64, hp, q0 + lo : q0 + 512],
                                start=True,
                                stop=True,
                            )
                        pend.append((ps, kc))
                        if len(pend) > 1:
                            do_exp_pv(*pend.pop(0))
                        pop_filler(1)
                    do_exp_pv(*pend.pop(0))

                    # evacuate po: PSUM -> stage (bf16, lh halves base-aligned
                    # with OT); rowsum rows -> rs_rows slots, bcast into rsb
                    st = stagep.tile([128, 512], bf16, tag="st", name=f"st{hp}_{qb}")
                    for lh in range(2):
                        s = 2 * hp + lh
                        nc.vector.tensor_copy(
                            st[64 * lh : 64 * lh + 64, :], po[lh][0:64, :]
                        )
                        nc.vector.tensor_copy(
                            rs_rows[32 * (s % 4) : 32 * (s % 4) + 1, s // 4, :],
                            po[lh][64:65, :],
                        )
                    stages[hp] = st

                def emit_norm(qb, stg, rs_rows):
                    """Normalize round qb's OT: approx-recip the rowsums,
                    cast to bf16, broadcast via DMA, multiply."""
                    q_sl = slice(qb * 512, (qb + 1) * 512)
                    rr = rs_rows.rearrange("p a q -> p (a q)")
                    nc.vector.reciprocal_approx_fast(rr[:, :], rr[:, :])
                    rs16 = stagep.tile(
                        [97, 2, 512], bf16, tag="rs16", bufs=2, name=f"rs16_{qb}"
                    )
                    nc.vector.tensor_copy(rs16[:], rs_rows[:])
                    for hp in range(4):
                        for lh in range(2):
                            s = 2 * hp + lh
                            row = rs16[32 * (s % 4) : 32 * (s % 4) + 1, s // 4, :]
                            nc.gpsimd.dma_start(
                                rsb[64 * lh : 64 * lh + 64, hp, :],
                                bass.AP(
                                    tensor=row.tensor,
                                    offset=row.offset,
                                    ap=[list(row.ap[0]), [0, 64], [1, 512]],
                                ),
                            )
                    for hp in range(4):
                        nc.vector.tensor_mul(
                            OT_sb[:, hp, q_sl],
                            stg[hp][:, :],
                            rsb[:, hp, :],
                        )

                prev_rs = [None]
                for qb in range(nqb):
                    if qb >= 1:
                        emit_norm(qb - 1, dict(stages), prev_rs[0])
                    # filler for this round: projections + V for round qb+1,
                    # output projection of round qb-1 (normalized just above)
                    if qb + 1 < nqb:
                        for hp in range(4):
                            filler.append(
                                lambda rt=4 + hp, nt=qb + 1: emit_proj(rt, nt)
                            )
                        for hp in range(4):
                            filler.append(lambda rt=hp, nt=qb + 1: emit_proj(rt, nt))
                        for tt in range(4 * qb + 4, 4 * qb + 8):
                            filler.append(lambda tt=tt: emit_v(tt))
                    if qb >= 1:
                        for tt in range(4 * (qb - 1), 4 * qb):
                            for nt in range(2):
                                filler.append(lambda tt=tt, nt=nt: emit_y(tt, nt))
                    rs_rows = stagep.tile(
                        [97, 2, 512], f32, tag="rs", bufs=2, name=f"rs{qb}"
                    )
                    prev_rs[0] = rs_rows
                    for hp in range(4):
                        attention(hp, qb, rs_rows)

                # tail: drain filler, normalize last round, project it
                pop_filler(len(filler))
                emit_norm(nqb - 1, dict(stages), prev_rs[0])
                for tt in range(4 * (nqb - 1), 4 * nqb):
                    for nt in range(2):
                        emit_y(tt, nt)

                if dbg:
                    nc.sync.dma_start(d_qkT[:], qkT_sb[:])
                    nc.sync.dma_start(d_V[:], V_sb[:])
                    nc.sync.dma_start(d_OT[:], OT_sb[:])

    nc.compile()
    return nc


def get_nc(T=2048, mm_dt="bf16", dbg=False):
    key = (T, dbg)
    if key not in _cache:
        _cache[key] = _build(T, dbg)
    return _cache[key]


def make_in_maps(x, qkv_w, qkv_b, proj_w, proj_b):
    import ml_dtypes

    bf = ml_dtypes.bfloat16
    B, T, _ = x.shape
    f = np.float32
    # S^T blocks are [key, query]: keep k <= q  ->  upper triangle
    tril = np.triu(np.ones((128, 128), f)).astype(bf)
    in_maps = []
    for i in range(B * 2):
        b, g = i // 2, i % 2
        sl = slice(g * 512, (g + 1) * 512)
        wq = qkv_w[0 * C : 1 * C][sl] * (1.0 / 8.0)
        wk = qkv_w[1 * C : 2 * C][sl]
        wv = qkv_w[2 * C : 3 * C][sl]
        # [rt, 128p, c, 128] -> partition-major [128p, rt, c, 128]
        wqk4 = np.stack(
            [
                np.concatenate([wq, wk], 0)
                .T[:, rt * 128 : (rt + 1) * 128]
                .reshape(C // 128, 128, 128)
                .transpose(1, 0, 2)
                for rt in range(C // 128)
            ]
        ).transpose(1, 0, 2, 3)
        in_maps.append(
            {
                "xT": np.ascontiguousarray(x[b].T).astype(bf),
                "wqkT": np.ascontiguousarray(wqk4).astype(bf),
                "wvT": np.ascontiguousarray(wv.T).astype(bf),
                "woT": np.ascontiguousarray(proj_w[:, sl].T).astype(bf),
                "qkb": np.concatenate(
                    [qkv_b[0 * C : 1 * C][sl] * (1.0 / 8.0), qkv_b[1 * C : 2 * C][sl]]
                ).astype(f),
                "vb": qkv_b[2 * C : 3 * C][sl].astype(bf),
                "tril": tril,
            }
        )
    return in_maps


def kernel(x, qkv_w, qkv_b, proj_w, proj_b, mm_dt="bf16", trace=False, tmpdir=None):
    from concourse.bass_utils import run_bass_kernel_spmd

    x = np.asarray(x, np.float32)
    qkv_w = np.asarray(qkv_w, np.float32)
    qkv_b = np.asarray(qkv_b, np.float32)
    proj_w = np.asarray(proj_w, np.float32)
    proj_b = np.asarray(proj_b, np.float32)

    B, T, _ = x.shape
    nc = get_nc(T)
    in_maps = make_in_maps(x, qkv_w, qkv_b, proj_w, proj_b)
    res = run_bass_kernel_spmd(
        nc, in_maps, list(range(len(in_maps))), trace=trace, tmpdir=tmpdir
    )
    out = np.empty((B, T, C), np.float32)
    for b in range(B):
        out[b] = res.results[2 * b]["y"] + res.results[2 * b + 1]["y"] + proj_b
    kernel.last_result = res
    return out


# revision 12
# speedup vs baseline: 1.3181x; 1.0272x over previous
"""Causal self-attention Trainium2 kernel (8-core SPMD), v2.

Reference: y = softmax(mask(q k^T / sqrt(dh))) v -> proj, with
x [B=4, T=2048, C=1024], H=16 heads, dh=64.

Sharding: core i handles batch b = i//2 and head-group g = i%2 (8 heads).
Each core computes a partial y (its heads' contribution to the output
projection); the host sums the two partials per batch and adds proj_b.

v2 design (vs v1): all matmul operands in bf16 (2 cols/cycle PE stream +
fast weight load), true-causal trimming of S/exp/PV, the two heads of a
pair share one exp instruction, and the whole kernel is a single
software-pipelined stream: QKV/V projections, attention, normalization
and the output projection are interleaved so the PE fills the gaps left
by the ACT-bound exp stream.

Per-core schedule:
  startup: const DMAs, weight DMAs, x columns, PE warmup (HAM
           un-throttle), exp-table preload, V ones-columns, projections
           for round 0
  round r (query block of 512 queries):
    normalize round r-1 (reciprocal + OT muls), then per head pair hp:
      pipelined kc chunks: S^T (PSUM [128, 2x512], trimmed), exp (ACT,
      one inst both heads), tril mask (DVE, diagonal), PV accumulate
      (PSUM [65,512]; V ones-column makes row 64 the softmax rowsum)
      + 1 filler tile per kc step: QK projection (round r+1), V tiles,
      output projection (round r-1)
      po evac: DVE [65,512] PSUM -> stage SBUF bf16; rowsum row
      broadcast-DMA'd into rsb [64,512]
  tail: normalize + output projection of the last round
"""

import numpy as np

C = 1024
HLOC = 8
DH = 64
QB = 512  # query block
KC = 128  # key chunk (PSUM partition dim)

_cache = {}


def _build(T, dbg=False):
    import concourse.bass as bass
    import concourse.tile as tile
    from concourse import bacc, mybir

    f32 = mybir.dt.float32
    bf16 = mybir.dt.bfloat16
    nqb = T // QB          # 4 rounds
    ctiles = C // 128      # 8
    ttiles = T // 128      # 16

    nc = bacc.Bacc("TRN2", target_bir_lowering=False, debug=False)

    xT = nc.dram_tensor("xT", [C, T], bf16, kind="ExternalInput")
    wqkT = nc.dram_tensor("wqkT", [128, ctiles, ctiles, 128], bf16, kind="ExternalInput")
    wvT = nc.dram_tensor("wvT", [C, 512], bf16, kind="ExternalInput")
    woT = nc.dram_tensor("woT", [512, C], bf16, kind="ExternalInput")
    qkb = nc.dram_tensor("qkb", [128, 8], f32, kind="ExternalInput")
    vb = nc.dram_tensor("vb", [512], bf16, kind="ExternalInput")
    tril = nc.dram_tensor("tril", [128, 128], bf16, kind="ExternalInput")
    y = nc.dram_tensor("y", [T, C], f32, kind="ExternalOutput")
    if dbg:
        d_qkT = nc.dram_tensor("d_qkT", [128, ctiles, T], bf16, kind="ExternalOutput")
        d_V = nc.dram_tensor("d_V", [128, ttiles, HLOC * 65], bf16, kind="ExternalOutput")
        d_OT = nc.dram_tensor("d_OT", [128, 4, T], bf16, kind="ExternalOutput")

    Exp = mybir.ActivationFunctionType.Exp
    Copy = mybir.ActivationFunctionType.Copy

    with nc.allow_low_precision("attention tolerates bf16 (rel tol 2e-2)"):
        with tile.TileContext(nc) as tc:
            with (
                tc.tile_pool(name="persist", bufs=1) as persist,
                tc.tile_pool(name="consts", bufs=1) as consts,
                tc.tile_pool(name="stage", bufs=16) as stagep,
                tc.tile_pool(name="pexp", bufs=3) as pexp,
                tc.tile_pool(name="yp", bufs=4) as yp,
                tc.tile_pool(name="pj", bufs=2, space="PSUM") as pj,
                tc.tile_pool(name="psS", bufs=2, space="PSUM") as psS,
                tc.tile_pool(name="psO", bufs=2, space="PSUM") as psO,
            ):
                qkT_sb = persist.tile([128, ctiles, T], bf16)
                V_sb = persist.tile([128, ttiles, HLOC * 65], bf16)
                xT_sb = persist.tile([128, ctiles, T], bf16)
                wqk_sb = persist.tile([128, ctiles, ctiles, 128], bf16)
                wvT_sb = persist.tile([128, ctiles, 512], bf16)
                woT_sb = persist.tile([128, 4, C], bf16)
                OT_sb = persist.tile([128, 4, T], bf16)
                rsb = persist.tile([128, 4, QB], bf16)
                tril_sb = consts.tile([128, 128], bf16)
                qkb_sb = consts.tile([128, ctiles], f32)
                vb_sb = consts.tile([128, 512], bf16)
                scr = consts.tile([1, 8], f32)

                # ---- startup DMAs (spread across queues/engines) ----
                nc.sync.dma_start(tril_sb[:], tril[:])
                nc.scalar.dma_start(qkb_sb[:], qkb[:])
                nc.gpsimd.dma_start(wqk_sb[:], wqkT[:])
                nc.gpsimd.dma_start(wvT_sb[:], wvT.rearrange("(c p) v -> p c v", p=128))
                nc.gpsimd.dma_start(woT_sb[:], woT.rearrange("(c p) o -> p c o", p=128))
                xT_r = xT.rearrange("(c p) t -> p c t", p=128)

                def dma_x(nt, c, eng):
                    eng.dma_start(
                        xT_sb[:, c, nt * 512 : (nt + 1) * 512],
                        xT_r[:, c, nt * 512 : (nt + 1) * 512],
                    )

                for c in range(4):
                    dma_x(0, c, nc.sync)
                for c in range(4, 8):
                    dma_x(0, c, nc.scalar)
                for c in range(ctiles):
                    dma_x(1, c, nc.sync)
                vb_ap = vb[:]
                nc.scalar.dma_start(
                    vb_sb[:],
                    bass.AP(
                        tensor=vb_ap.tensor, offset=vb_ap.offset, ap=[[0, 128], [1, 512]]
                    ),
                )
                for nt in (2, 3):
                    for c in range(ctiles):
                        dma_x(nt, c, nc.gpsimd)

                # exp table preload (off the hot path)
                nc.scalar.activation(scr[:, 0:8], tril_sb[0:1, 0:8], Exp)

                # V ones-columns (col 64 of each head's 65-wide slot)
                v_grp = V_sb.rearrange("p t (h c) -> p t h c", c=65)
                nc.scalar.activation(
                    v_grp[:, :, :, 64:65],
                    tril_sb[:, 0 : ttiles * HLOC].rearrange(
                        "p (a b c) -> p a b c", a=ttiles, b=HLOC, c=1
                    ),
                    Copy,
                    bias=1.0,
                    scale=0.0,
                )

                # ---- PE warmup: un-throttle HAM (~4.5us of dummy matmuls),
                # depends only on tril (first DMA)
                for w in range(40):
                    wp = pj.tile([128, 512], f32, tag="pj", name=f"wp{w}")
                    nc.tensor.matmul(
                        wp[:, 0:128], tril_sb[:], tril_sb[:], start=True, stop=True
                    )

                # ---------- filler emission machinery ----------
                def emit_proj(rt, nt):
                    """One QK-projection output tile [128, 512] -> qkT_sb."""
                    ps = pj.tile([128, 512], f32, tag="pj", name=f"pj{rt}_{nt}")
                    for c in range(ctiles):
                        nc.tensor.matmul(
                            ps[:],
                            wqk_sb[:, rt, c, :],
                            xT_sb[:, c, nt * 512 : (nt + 1) * 512],
                            start=(c == 0),
                            stop=(c == ctiles - 1),
                        )
                    nc.vector.tensor_scalar_add(
                        qkT_sb[:, rt, nt * 512 : (nt + 1) * 512],
                        ps[:],
                        qkb_sb[:, rt : rt + 1],
                    )

                def emit_v(tt):
                    """One V tile [128 keys, 512 v-cols] -> V_sb (+bias)."""
                    ps = pj.tile([128, 512], f32, tag="pj", name=f"pv{tt}")
                    for c in range(ctiles):
                        nc.tensor.matmul(
                            ps[:],
                            xT_sb[:, c, tt * 128 : (tt + 1) * 128],
                            wvT_sb[:, c, :],
                            start=(c == 0),
                            stop=(c == ctiles - 1),
                        )
                    nc.vector.tensor_add(
                        v_grp[:, tt, :, 0:64],
                        ps.rearrange("p (h c) -> p h c", c=64),
                        vb_sb.rearrange("p (h c) -> p h c", c=64),
                    )

                def emit_y(tt, nt):
                    """One output-projection tile y[tt*128:, nt*512:]."""
                    ps = pj.tile([128, 512], f32, tag="pj", name=f"py{tt}_{nt}")
                    for c4 in range(4):
                        nc.tensor.matmul(
                            ps[:],
                            OT_sb[:, c4, tt * 128 : (tt + 1) * 128],
                            woT_sb[:, c4, nt * 512 : (nt + 1) * 512],
                            start=(c4 == 0),
                            stop=(c4 == 3),
                        )
                    yt = yp.tile([128, 512], f32, tag="yt", name=f"yt{tt}_{nt}")
                    nc.vector.tensor_copy(yt[:], ps[:])
                    nc.sync.dma_start(
                        y[tt * 128 : (tt + 1) * 128, nt * 512 : (nt + 1) * 512],
                        yt[:],
                    )

                filler = []

                def pop_filler(n=1):
                    for _ in range(n):
                        if filler:
                            filler.pop(0)()

                # pre-round work: projections for round 0 (K first), V 0..3
                for hp in range(4):
                    emit_proj(4 + hp, 0)
                for hp in range(4):
                    emit_proj(hp, 0)
                for tt in range(4):
                    emit_v(tt)

                # ---------- main rounds ----------
                stages = {}  # (hp, lh) -> stage tile of current round

                def attention(hp, qb, rs_rows):
                    nkc = 4 * (qb + 1)
                    q0 = qb * 512
                    po = [
                        psO.tile([65, 512], f32, tag="po", name=f"po{hp}_{qb}_{i}")
                        for i in range(2)
                    ]
                    pend = []  # psS tiles awaiting exp+PV: (ps, kc)

                    def do_exp_pv(ps, kc):
                        lo = max(kc * 128 - q0, 0)
                        ps3 = ps.rearrange("p (l q) -> p l q", l=2)
                        pt = pexp.tile([128, 2, 512], bf16, tag="pt", name=f"pt{hp}_{qb}_{kc}")
                        nc.scalar.activation(
                            pt[:, :, lo:512], ps3[:, :, lo:512], Exp
                        )
                        if kc * 128 >= q0:  # diagonal chunk: tril mask
                            for lh in range(2):
                                nc.vector.tensor_mul(
                                    pt[:, lh, lo : lo + 128],
                                    pt[:, lh, lo : lo + 128],
                                    tril_sb[:],
                                )
                        for lh in range(2):
                            nc.tensor.matmul(
                                po[lh][:, lo:512],
                                V_sb[:, kc, (2 * hp + lh) * 65 : (2 * hp + lh) * 65 + 65],
                                pt[:, lh, lo:512],
                                start=(kc == 0),
                                stop=(kc == nkc - 1),
                            )

                    for kc in range(nkc):
                        lo = max(kc * 128 - q0, 0)
                        ps = psS.tile([128, 1024], f32, tag="ps", name=f"ps{hp}_{qb}_{kc}")
                        ps3 = ps.rearrange("p (l q) -> p l q", l=2)
                        for lh in range(2):
                            b0 = 64 * lh
                            nc.tensor.matmul(
                                ps3[:, lh, lo:512],
                                qkT_sb[b0 : b0 + 64, 4 + hp, kc * 128 : (kc + 1) * 128],
                                qkT_sb[b0 : b0 + 64, hp, q0 + lo : q0 + 512],
                                start=True,
                                stop=True,
                            )
                        pend.append((ps, kc))
                        if len(pend) > 1:
                            do_exp_pv(*pend.pop(0))
                        pop_filler(1)
                    do_exp_pv(*pend.pop(0))

                    # evacuate po: PSUM -> stage (bf16, lh halves base-aligned
                    # with OT); rowsum rows -> rs_rows slots, bcast into rsb
                    st = stagep.tile([128, 512], bf16, tag="st", name=f"st{hp}_{qb}")
                    for lh in range(2):
                        s = 2 * hp + lh
                        nc.vector.tensor_copy(
                            st[64 * lh : 64 * lh + 64, :], po[lh][0:64, :]
                        )
                        nc.vector.tensor_copy(
                            rs_rows[32 * (s % 4) : 32 * (s % 4) + 1, s // 4, :],
                            po[lh][64:65, :],
                        )
                    stages[hp] = st

                def emit_norm_a(qb, rs_rows):
                    """Recip the rowsums, cast to bf16, broadcast into rsb."""
                    rr = rs_rows.rearrange("p a q -> p (a q)")
                    nc.vector.reciprocal_approx_fast(rr[:, :], rr[:, :])
                    rs16 = stagep.tile(
                        [97, 2, 512], bf16, tag="rs16", bufs=2, name=f"rs16_{qb}"
                    )
                    nc.vector.tensor_copy(rs16[:], rs_rows[:])
                    for hp in range(4):
                        for lh in range(2):
                            s = 2 * hp + lh
                            row = rs16[32 * (s % 4) : 32 * (s % 4) + 1, s // 4, :]
                            nc.gpsimd.dma_start(
                                rsb[64 * lh : 64 * lh + 64, hp, :],
                                bass.AP(
                                    tensor=row.tensor,
                                    offset=row.offset,
                                    ap=[list(row.ap[0]), [0, 64], [1, 512]],
                                ),
                            )

                def emit_norm_b(qb, stg):
                    """OT = stage * rsb (after the broadcasts land)."""
                    q_sl = slice(qb * 512, (qb + 1) * 512)
                    for hp in range(4):
                        nc.vector.tensor_mul(
                            OT_sb[:, hp, q_sl],
                            stg[hp][:, :],
                            rsb[:, hp, :],
                        )

                prev_rs = [None]
                for qb in range(nqb):
                    if qb >= 1:
                        emit_norm_a(qb - 1, prev_rs[0])
                    # filler for this round: projections + V for round qb+1,
                    # normalize-muls then output projection of round qb-1
                    if qb + 1 < nqb:
                        for hp in range(4):
                            filler.append(
                                lambda rt=4 + hp, nt=qb + 1: emit_proj(rt, nt)
                            )
                    if qb >= 1:
                        filler.append(
                            lambda qb=qb, stg=dict(stages): emit_norm_b(qb - 1, stg)
                        )
                    if qb + 1 < nqb:
                        for hp in range(4):
                            filler.append(lambda rt=hp, nt=qb + 1: emit_proj(rt, nt))
                        for tt in range(4 * qb + 4, 4 * qb + 8):
                            filler.append(lambda tt=tt: emit_v(tt))
                    if qb >= 1:
                        for tt in range(4 * (qb - 1), 4 * qb):
                            for nt in range(2):
                                filler.append(lambda tt=tt, nt=nt: emit_y(tt, nt))
                    rs_rows = stagep.tile(
                        [97, 2, 512], f32, tag="rs", bufs=2, name=f"rs{qb}"
                    )
                    prev_rs[0] = rs_rows
                    for hp in range(4):
                        attention(hp, qb, rs_rows)

                # tail: drain filler, normalize last round, project it
                pop_filler(len(filler))
                emit_norm_a(nqb - 1, prev_rs[0])
                emit_norm_b(nqb - 1, dict(stages))
                for tt in range(4 * (nqb - 1), 4 * nqb):
                    for nt in range(2):
                        emit_y(tt, nt)

                if dbg:
                    nc.sync.dma_start(d_qkT[:], qkT_sb[:])
                    nc.sync.dma_start(d_V[:], V_sb[:])
                    nc.sync.dma_start(d_OT[:], OT_sb[:])

    nc.compile()
    return nc


def get_nc(T=2048, mm_dt="bf16", dbg=False):
    key = (T, dbg)
    if key not in _cache:
        _cache[key] = _build(T, dbg)
    return _cache[key]


def make_in_maps(x, qkv_w, qkv_b, proj_w, proj_b):
    import ml_dtypes

    bf = ml_dtypes.bfloat16
    B, T, _ = x.shape
    f = np.float32
    # S^T blocks are [key, query]: keep k <= q  ->  upper triangle
    tril = np.triu(np.ones((128, 128), f)).astype(bf)
    in_maps = []
    for i in range(B * 2):
        b, g = i // 2, i % 2
        sl = slice(g * 512, (g + 1) * 512)
        wq = qkv_w[0 * C : 1 * C][sl] * (1.0 / 8.0)
        wk = qkv_w[1 * C : 2 * C][sl]
        wv = qkv_w[2 * C : 3 * C][sl]
        # [rt, 128p, c, 128] -> partition-major [128p, rt, c, 128]
        wqk4 = np.stack(
            [
                np.concatenate([wq, wk], 0)
                .T[:, rt * 128 : (rt + 1) * 128]
                .reshape(C // 128, 128, 128)
                .transpose(1, 0, 2)
                for rt in range(C // 128)
            ]
        ).transpose(1, 0, 2, 3)
        in_maps.append(
            {
                "xT": np.ascontiguousarray(x[b].T).astype(bf),
                "wqkT": np.ascontiguousarray(wqk4).astype(bf),
                "wvT": np.ascontiguousarray(wv.T).astype(bf),
                "woT": np.ascontiguousarray(proj_w[:, sl].T).astype(bf),
                "qkb": np.ascontiguousarray(
                    np.concatenate(
                        [qkv_b[0 * C : 1 * C][sl] * (1.0 / 8.0), qkv_b[1 * C : 2 * C][sl]]
                    ).reshape(8, 128).T
                ).astype(f),
                "vb": qkv_b[2 * C : 3 * C][sl].astype(bf),
                "tril": tril,
            }
        )
    return in_maps


def kernel(x, qkv_w, qkv_b, proj_w, proj_b, mm_dt="bf16", trace=False, tmpdir=None):
    from concourse.bass_utils import run_bass_kernel_spmd

    x = np.asarray(x, np.float32)
    qkv_w = np.asarray(qkv_w, np.float32)
    qkv_b = np.asarray(qkv_b, np.float32)
    proj_w = np.asarray(proj_w, np.float32)
    proj_b = np.asarray(proj_b, np.float32)

    B, T, _ = x.shape
    nc = get_nc(T)
    in_maps = make_in_maps(x, qkv_w, qkv_b, proj_w, proj_b)
    res = run_bass_kernel_spmd(
        nc, in_maps, list(range(len(in_maps))), trace=trace, tmpdir=tmpdir
    )
    out = np.empty((B, T, C), np.float32)
    for b in range(B):
        out[b] = res.results[2 * b]["y"] + res.results[2 * b + 1]["y"] + proj_b
    kernel.last_result = res
    return out


# revision 13
# speedup vs baseline: 1.3413x; 1.0176x over previous
"""Causal self-attention Trainium2 kernel (8-core SPMD), v2.

Reference: y = softmax(mask(q k^T / sqrt(dh))) v -> proj, with
x [B=4, T=2048, C=1024], H=16 heads, dh=64.

Sharding: core i handles batch b = i//2 and head-group g = i%2 (8 heads).
Each core computes a partial y (its heads' contribution to the output
projection); the host sums the two partials per batch and adds proj_b.

v2 design (vs v1): all matmul operands in bf16 (2 cols/cycle PE stream +
fast weight load), true-causal trimming of S/exp/PV, the two heads of a
pair share one exp instruction, and the whole kernel is a single
software-pipelined stream: QKV/V projections, attention, normalization
and the output projection are interleaved so the PE fills the gaps left
by the ACT-bound exp stream.

Per-core schedule:
  startup: const DMAs, weight DMAs, x columns, PE warmup (HAM
           un-throttle), exp-table preload, V ones-columns, projections
           for round 0
  round r (query block of 512 queries):
    normalize round r-1 (reciprocal + OT muls), then per head pair hp:
      pipelined kc chunks: S^T (PSUM [128, 2x512], trimmed), exp (ACT,
      one inst both heads), tril mask (DVE, diagonal), PV accumulate
      (PSUM [65,512]; V ones-column makes row 64 the softmax rowsum)
      + 1 filler tile per kc step: QK projection (round r+1), V tiles,
      output projection (round r-1)
      po evac: DVE [65,512] PSUM -> stage SBUF bf16; rowsum row
      broadcast-DMA'd into rsb [64,512]
  tail: normalize + output projection of the last round
"""

import numpy as np

C = 1024
HLOC = 8
DH = 64
QB = 512  # query block
KC = 128  # key chunk (PSUM partition dim)

_cache = {}


def _build(T, dbg=False):
    import concourse.bass as bass
    import concourse.tile as tile
    from concourse import bacc, mybir

    f32 = mybir.dt.float32
    bf16 = mybir.dt.bfloat16
    nqb = T // QB          # 4 rounds
    ctiles = C // 128      # 8
    ttiles = T // 128      # 16

    nc = bacc.Bacc("TRN2", target_bir_lowering=False, debug=False)

    xT = nc.dram_tensor("xT", [C, T], bf16, kind="ExternalInput")
    wqkT = nc.dram_tensor("wqkT", [128, ctiles, ctiles, 128], bf16, kind="ExternalInput")
    wvT = nc.dram_tensor("wvT", [C, 512], bf16, kind="ExternalInput")
    woT = nc.dram_tensor("woT", [512, C], bf16, kind="ExternalInput")
    qkb = nc.dram_tensor("qkb", [128, 8], f32, kind="ExternalInput")
    vb = nc.dram_tensor("vb", [512], bf16, kind="ExternalInput")
    tril = nc.dram_tensor("tril", [128, 128], bf16, kind="ExternalInput")
    y = nc.dram_tensor("y", [T, C], f32, kind="ExternalOutput")
    if dbg:
        d_qkT = nc.dram_tensor("d_qkT", [128, ctiles, T], bf16, kind="ExternalOutput")
        d_V = nc.dram_tensor("d_V", [128, ttiles, HLOC * 65], bf16, kind="ExternalOutput")
        d_OT = nc.dram_tensor("d_OT", [128, 4, T], bf16, kind="ExternalOutput")

    Exp = mybir.ActivationFunctionType.Exp
    Copy = mybir.ActivationFunctionType.Copy

    with nc.allow_low_precision("attention tolerates bf16 (rel tol 2e-2)"):
        with tile.TileContext(nc) as tc:
            with (
                tc.tile_pool(name="persist", bufs=1) as persist,
                tc.tile_pool(name="consts", bufs=1) as consts,
                tc.tile_pool(name="stage", bufs=16) as stagep,
                tc.tile_pool(name="pexp", bufs=3) as pexp,
                tc.tile_pool(name="yp", bufs=4) as yp,
                tc.tile_pool(name="pj", bufs=2, space="PSUM") as pj,
                tc.tile_pool(name="psS", bufs=2, space="PSUM") as psS,
                tc.tile_pool(name="psO", bufs=2, space="PSUM") as psO,
            ):
                qkT_sb = persist.tile([128, ctiles, T], bf16)
                V_sb = persist.tile([128, ttiles, HLOC * 65], bf16)
                xT_sb = persist.tile([128, ctiles, T], bf16)
                wqk_sb = persist.tile([128, ctiles, ctiles, 128], bf16)
                wvT_sb = persist.tile([128, ctiles, 512], bf16)
                woT_sb = persist.tile([128, 4, C], bf16)
                OT_sb = persist.tile([128, 4, T], bf16)
                rsb = persist.tile([128, 4, QB], bf16)
                tril_sb = consts.tile([128, 128], bf16)
                qkb_sb = consts.tile([128, ctiles], f32)
                vb_sb = consts.tile([128, 512], bf16)
                scr = consts.tile([1, 8], f32)

                # ---- startup DMAs (spread across queues/engines) ----
                nc.sync.dma_start(tril_sb[:], tril[:])
                nc.scalar.dma_start(qkb_sb[:], qkb[:])
                nc.gpsimd.dma_start(wqk_sb[:], wqkT[:])
                nc.gpsimd.dma_start(wvT_sb[:], wvT.rearrange("(c p) v -> p c v", p=128))
                nc.gpsimd.dma_start(woT_sb[:], woT.rearrange("(c p) o -> p c o", p=128))
                xT_r = xT.rearrange("(c p) t -> p c t", p=128)

                def dma_x(nt, c, eng):
                    eng.dma_start(
                        xT_sb[:, c, nt * 512 : (nt + 1) * 512],
                        xT_r[:, c, nt * 512 : (nt + 1) * 512],
                    )

                for c in range(4):
                    dma_x(0, c, nc.sync)
                for c in range(4, 8):
                    dma_x(0, c, nc.scalar)
                for c in range(ctiles):
                    dma_x(1, c, nc.sync)
                vb_ap = vb[:]
                nc.scalar.dma_start(
                    vb_sb[:],
                    bass.AP(
                        tensor=vb_ap.tensor, offset=vb_ap.offset, ap=[[0, 128], [1, 512]]
                    ),
                )
                for nt in (2, 3):
                    for c in range(ctiles):
                        dma_x(nt, c, nc.gpsimd)

                # exp table preload (off the hot path)
                nc.scalar.activation(scr[:, 0:8], tril_sb[0:1, 0:8], Exp)

                # V ones-columns (col 64 of each head's 65-wide slot)
                v_grp = V_sb.rearrange("p t (h c) -> p t h c", c=65)
                nc.scalar.activation(
                    v_grp[:, :, :, 64:65],
                    tril_sb[:, 0 : ttiles * HLOC].rearrange(
                        "p (a b c) -> p a b c", a=ttiles, b=HLOC, c=1
                    ),
                    Copy,
                    bias=1.0,
                    scale=0.0,
                )

                # ---- PE warmup: un-throttle HAM (~4.5us of dummy matmuls),
                # depends only on tril (first DMA)
                for w in range(40):
                    wp = pj.tile([128, 512], f32, tag="pj", name=f"wp{w}")
                    nc.tensor.matmul(
                        wp[:, 0:128], tril_sb[:], tril_sb[:], start=True, stop=True
                    )

                # ---------- filler emission machinery ----------
                def emit_proj(rt, nt):
                    """One QK-projection output tile [128, 512] -> qkT_sb."""
                    ps = pj.tile([128, 512], f32, tag="pj", name=f"pj{rt}_{nt}")
                    for c in range(ctiles):
                        nc.tensor.matmul(
                            ps[:],
                            wqk_sb[:, rt, c, :],
                            xT_sb[:, c, nt * 512 : (nt + 1) * 512],
                            start=(c == 0),
                            stop=(c == ctiles - 1),
                        )
                    nc.vector.tensor_scalar_add(
                        qkT_sb[:, rt, nt * 512 : (nt + 1) * 512],
                        ps[:],
                        qkb_sb[:, rt : rt + 1],
                    )

                def emit_v(tt):
                    """One V tile [128 keys, 512 v-cols] -> V_sb (+bias)."""
                    ps = pj.tile([128, 512], f32, tag="pj", name=f"pv{tt}")
                    for c in range(ctiles):
                        nc.tensor.matmul(
                            ps[:],
                            xT_sb[:, c, tt * 128 : (tt + 1) * 128],
                            wvT_sb[:, c, :],
                            start=(c == 0),
                            stop=(c == ctiles - 1),
                        )
                    nc.vector.tensor_add(
                        v_grp[:, tt, :, 0:64],
                        ps.rearrange("p (h c) -> p h c", c=64),
                        vb_sb.rearrange("p (h c) -> p h c", c=64),
                    )

                def emit_y(tt, nt):
                    """One output-projection tile y[tt*128:, nt*512:]."""
                    ps = pj.tile([128, 512], f32, tag="pj", name=f"py{tt}_{nt}")
                    for c4 in range(4):
                        nc.tensor.matmul(
                            ps[:],
                            OT_sb[:, c4, tt * 128 : (tt + 1) * 128],
                            woT_sb[:, c4, nt * 512 : (nt + 1) * 512],
                            start=(c4 == 0),
                            stop=(c4 == 3),
                        )
                    yt = yp.tile([128, 512], f32, tag="yt", name=f"yt{tt}_{nt}")
                    nc.vector.tensor_copy(yt[:], ps[:])
                    nc.sync.dma_start(
                        y[tt * 128 : (tt + 1) * 128, nt * 512 : (nt + 1) * 512],
                        yt[:],
                    )

                filler = []

                def pop_filler(n=1):
                    for _ in range(n):
                        if filler:
                            filler.pop(0)()

                # pre-round work: projections for round 0 (K first), V 0..3
                for hp in range(4):
                    emit_proj(4 + hp, 0)
                for hp in range(4):
                    emit_proj(hp, 0)
                for tt in range(4):
                    emit_v(tt)

                # ---------- main rounds ----------
                stages = {}  # (hp, lh) -> stage tile of current round

                def attention(hp, qb, rs_rows):
                    nkc = 4 * (qb + 1)
                    q0 = qb * 512
                    po = [
                        psO.tile([65, 512], f32, tag="po", name=f"po{hp}_{qb}_{i}")
                        for i in range(2)
                    ]
                    pend_exp = []  # psS tiles awaiting exp: (ps, kc)
                    pend_pv = []   # pt tiles awaiting PV: (pt, kc)

                    def do_exp(ps, kc):
                        lo = max(kc * 128 - q0, 0)
                        ps3 = ps.rearrange("p (l q) -> p l q", l=2)
                        pt = pexp.tile([128, 2, 512], bf16, tag="pt", name=f"pt{hp}_{qb}_{kc}")
                        nc.scalar.activation(
                            pt[:, :, lo:512], ps3[:, :, lo:512], Exp
                        )
                        if kc * 128 >= q0:  # diagonal chunk: tril mask
                            for lh in range(2):
                                nc.vector.tensor_mul(
                                    pt[:, lh, lo : lo + 128],
                                    pt[:, lh, lo : lo + 128],
                                    tril_sb[:],
                                )
                        return pt

                    def do_pv(pt, kc):
                        lo = max(kc * 128 - q0, 0)
                        for lh in range(2):
                            nc.tensor.matmul(
                                po[lh][:, lo:512],
                                V_sb[:, kc, (2 * hp + lh) * 65 : (2 * hp + lh) * 65 + 65],
                                pt[:, lh, lo:512],
                                start=(kc == 0),
                                stop=(kc == nkc - 1),
                            )

                    for kc in range(nkc):
                        lo = max(kc * 128 - q0, 0)
                        ps = psS.tile([128, 1024], f32, tag="ps", name=f"ps{hp}_{qb}_{kc}")
                        ps3 = ps.rearrange("p (l q) -> p l q", l=2)
                        for lh in range(2):
                            b0 = 64 * lh
                            nc.tensor.matmul(
                                ps3[:, lh, lo:512],
                                qkT_sb[b0 : b0 + 64, 4 + hp, kc * 128 : (kc + 1) * 128],
                                qkT_sb[b0 : b0 + 64, hp, q0 + lo : q0 + 512],
                                start=True,
                                stop=True,
                            )
                        pend_exp.append((ps, kc))
                        if len(pend_exp) > 1:
                            ps2, kc2 = pend_exp.pop(0)
                            pend_pv.append((do_exp(ps2, kc2), kc2))
                        if len(pend_pv) > 1:
                            do_pv(*pend_pv.pop(0))
                        pop_filler(1)
                    pend_pv.append((do_exp(*pend_exp.pop(0)), nkc - 1))
                    while pend_pv:
                        do_pv(*pend_pv.pop(0))

                    # evacuate po: PSUM -> stage (bf16, lh halves base-aligned
                    # with OT); rowsum rows -> rs_rows slots, bcast into rsb
                    st = stagep.tile([128, 512], bf16, tag="st", name=f"st{hp}_{qb}")
                    for lh in range(2):
                        s = 2 * hp + lh
                        nc.vector.tensor_copy(
                            st[64 * lh : 64 * lh + 64, :], po[lh][0:64, :]
                        )
                        nc.vector.tensor_copy(
                            rs_rows[32 * (s % 4) : 32 * (s % 4) + 1, s // 4, :],
                            po[lh][64:65, :],
                        )
                    stages[hp] = st

                def emit_norm_a(qb, rs_rows):
                    """Recip the rowsums, cast to bf16, broadcast into rsb."""
                    rr = rs_rows.rearrange("p a q -> p (a q)")
                    nc.vector.reciprocal_approx_fast(rr[:, :], rr[:, :])
                    rs16 = stagep.tile(
                        [97, 2, 512], bf16, tag="rs16", bufs=2, name=f"rs16_{qb}"
                    )
                    nc.vector.tensor_copy(rs16[:], rs_rows[:])
                    for hp in range(4):
                        for lh in range(2):
                            s = 2 * hp + lh
                            row = rs16[32 * (s % 4) : 32 * (s % 4) + 1, s // 4, :]
                            nc.gpsimd.dma_start(
                                rsb[64 * lh : 64 * lh + 64, hp, :],
                                bass.AP(
                                    tensor=row.tensor,
                                    offset=row.offset,
                                    ap=[list(row.ap[0]), [0, 64], [1, 512]],
                                ),
                            )

                def emit_norm_b(qb, stg):
                    """OT = stage * rsb (after the broadcasts land)."""
                    q_sl = slice(qb * 512, (qb + 1) * 512)
                    for hp in range(4):
                        nc.vector.tensor_mul(
                            OT_sb[:, hp, q_sl],
                            stg[hp][:, :],
                            rsb[:, hp, :],
                        )

                prev_rs = [None]
                for qb in range(nqb):
                    if qb >= 1:
                        emit_norm_a(qb - 1, prev_rs[0])
                    # filler for this round: projections + V for round qb+1,
                    # normalize-muls then output projection of round qb-1
                    if qb + 1 < nqb:
                        for hp in range(4):
                            filler.append(
                                lambda rt=4 + hp, nt=qb + 1: emit_proj(rt, nt)
                            )
                    if qb >= 1:
                        filler.append(
                            lambda qb=qb, stg=dict(stages): emit_norm_b(qb - 1, stg)
                        )
                    if qb + 1 < nqb:
                        for hp in range(4):
                            filler.append(lambda rt=hp, nt=qb + 1: emit_proj(rt, nt))
                        for tt in range(4 * qb + 4, 4 * qb + 8):
                            filler.append(lambda tt=tt: emit_v(tt))
                    if qb >= 1:
                        for tt in range(4 * (qb - 1), 4 * qb):
                            for nt in range(2):
                                filler.append(lambda tt=tt, nt=nt: emit_y(tt, nt))
                    rs_rows = stagep.tile(
                        [97, 2, 512], f32, tag="rs", bufs=2, name=f"rs{qb}"
                    )
                    prev_rs[0] = rs_rows
                    for hp in range(4):
                        attention(hp, qb, rs_rows)

                # tail: drain filler, normalize last round, project it
                pop_filler(len(filler))
                emit_norm_a(nqb - 1, prev_rs[0])
                emit_norm_b(nqb - 1, dict(stages))
                for tt in range(4 * (nqb - 1), 4 * nqb):
                    for nt in range(2):
                        emit_y(tt, nt)

                if dbg:
                    nc.sync.dma_start(d_qkT[:], qkT_sb[:])
                    nc.sync.dma_start(d_V[:], V_sb[:])
                    nc.sync.dma_start(d_OT[:], OT_sb[:])

    nc.compile()
    return nc


def get_nc(T=2048, mm_dt="bf16", dbg=False):
    key = (T, dbg)
    if key not in _cache:
        _cache[key] = _build(T, dbg)
    return _cache[key]


def make_in_maps(x, qkv_w, qkv_b, proj_w, proj_b):
    import ml_dtypes

    bf = ml_dtypes.bfloat16
    B, T, _ = x.shape
    f = np.float32
    # S^T blocks are [key, query]: keep k <= q  ->  upper triangle
    tril = np.triu(np.ones((128, 128), f)).astype(bf)
    in_maps = []
    for i in range(B * 2):
        b, g = i // 2, i % 2
        sl = slice(g * 512, (g + 1) * 512)
        wq = qkv_w[0 * C : 1 * C][sl] * (1.0 / 8.0)
        wk = qkv_w[1 * C : 2 * C][sl]
        wv = qkv_w[2 * C : 3 * C][sl]
        # [rt, 128p, c, 128] -> partition-major [128p, rt, c, 128]
        wqk4 = np.stack(
            [
                np.concatenate([wq, wk], 0)
                .T[:, rt * 128 : (rt + 1) * 128]
                .reshape(C // 128, 128, 128)
                .transpose(1, 0, 2)
                for rt in range(C // 128)
            ]
        ).transpose(1, 0, 2, 3)
        in_maps.append(
            {
                "xT": np.ascontiguousarray(x[b].T).astype(bf),
                "wqkT": np.ascontiguousarray(wqk4).astype(bf),
                "wvT": np.ascontiguousarray(wv.T).astype(bf),
                "woT": np.ascontiguousarray(proj_w[:, sl].T).astype(bf),
                "qkb": np.ascontiguousarray(
                    np.concatenate(
                        [qkv_b[0 * C : 1 * C][sl] * (1.0 / 8.0), qkv_b[1 * C : 2 * C][sl]]
                    ).reshape(8, 128).T
                ).astype(f),
                "vb": qkv_b[2 * C : 3 * C][sl].astype(bf),
                "tril": tril,
            }
        )
    return in_maps


def kernel(x, qkv_w, qkv_b, proj_w, proj_b, mm_dt="bf16", trace=False, tmpdir=None):
    from concourse.bass_utils import run_bass_kernel_spmd

    x = np.asarray(x, np.float32)
    qkv_w = np.asarray(qkv_w, np.float32)
    qkv_b = np.asarray(qkv_b, np.float32)
    proj_w = np.asarray(proj_w, np.float32)
    proj_b = np.asarray(proj_b, np.float32)

    B, T, _ = x.shape
    nc = get_nc(T)
    in_maps = make_in_maps(x, qkv_w, qkv_b, proj_w, proj_b)
    res = run_bass_kernel_spmd(
        nc, in_maps, list(range(len(in_maps))), trace=trace, tmpdir=tmpdir
    )
    out = np.empty((B, T, C), np.float32)
    for b in range(B):
        out[b] = res.results[2 * b]["y"] + res.results[2 * b + 1]["y"] + proj_b
    kernel.last_result = res
    return out
